# revision 11
# baseline (speedup 1.0000x reference)
"""Single-head attention (batch 8, seq 4096, embed 1024, head 64) on 8 TRN2
NeuronCores, data-parallel over batch (one batch element per core).

Per-core pipeline (bf16 matmul operands, fp32 PSUM accumulation everywhere):
  1. Load x [4096, 1024] (pre-cast to bf16 on host), PE-transpose to
     x^T [1024, 4096] in s-blocks.
  2. Projections vs x^T (contraction on partitions):
       [Wq|Wv] pass -> QV tile [128, s]: rows 0:64 = Q^T, rows 64:128 = V^T
       [Wk|Wk] pass -> Kt tile [128, s]: rows 0:64 = K^T
     V^T slices are PE-transposed back to V-natural [128, 65] tiles with a
     ones column appended (softmax denominator trick).
  3. Attention in S^T orientation (scores transposed: [sk, sq]), so the
     P @ V contraction needs no transposes of the [4096, 4096] matrix:
       S^T chunk [128 sk, 1024 sq] = (K^T slice).T @ Q^T   (K=64 contraction)
       P^T = exp(S^T / 8) on ScalarE (fp32 PSUM -> bf16 SBUF); no max
       subtraction needed: scores ~N(0, 0.33^2), exp is safe in fp32.
       O^T [65, sq] += [V_j | 1].T @ P^T_j over all 32 sk chunks; row 64
       accumulates the softmax denominator (fp32 PSUM).
  4. Epilogue per sq block: reciprocal of the denominator row, gpsimd
     partition-broadcast, normalize in O^T layout, PE-transpose to
     O-natural, DMA out (fp32).
"""

import numpy as np

import concourse.bass as bass
import concourse.mybir as mybir
import concourse.tile as tile
from concourse import bacc
from concourse.bass_utils import run_bass_kernel_spmd

S = 4096  # sequence length (per core)
E = 1024  # embed dim
H = 64  # head size
B = 8  # batch == number of cores

SB = 1024  # prologue s-block
NSB = S // SB
QB = 1024  # attention sq-block
NQB = S // QB
CH = 128  # sk chunk
NCH = S // CH

f32 = mybir.dt.float32
bf16 = mybir.dt.bfloat16
EXP = mybir.ActivationFunctionType.Exp

_cache = {}


def _emit_iteration(nc, tc, ps, pp, consts):
    eye, wqv, wkk, x_d, ones_d, out_d = consts
    EC = E // 128

    qv_tiles = []  # [128, SB] per s-block: rows 0:64 Q^T, 64:128 V^T
    kt_tiles = []  # [128, SB] per s-block: rows 0:64 K^T
    for sb in range(NSB):
        qv_tiles.append(pp.tile([128, SB], bf16, tag=f"qv{sb}", name=f"qv{sb}"))
        kt_tiles.append(pp.tile([128, SB], bf16, tag=f"kt{sb}", name=f"kt{sb}"))
    v_tiles = []  # [128, 65] V natural + ones column, per sk chunk
    for j in range(NCH):
        v_tiles.append(pp.tile([128, 65], bf16, tag=f"v{j}", name=f"v{j}"))

    # ---------------- prologue: x^T (DMA transpose) + projections ----------------
    for j in range(NCH):
        nc.sync.dma_start(out=v_tiles[j][:, 64:65], in_=ones_d[:])
    with tc.tile_pool(name="xt", bufs=1) as xtp:
        xt_blk = []
        for c in range(EC):
            xt_c = xtp.tile([128, S], bf16, tag=f"xt{c}", name=f"xt{c}")
            nc.sync.dma_start_transpose(xt_c[:], x_d[:, c * 128 : (c + 1) * 128])
            xt_blk.append(xt_c)
        for sb in range(NSB):
            s0 = sb * SB
            # projections: QV pass and KK pass
            for w_tiles, dst in ((wqv, qv_tiles[sb]), (wkk, kt_tiles[sb])):
                pj = ps.tile([128, SB], f32, tag="b")
                for half in range(SB // 512):
                    fsl = slice(half * 512, (half + 1) * 512)
                    for c in range(EC):
                        nc.tensor.matmul(
                            pj[:, fsl],
                            w_tiles[c][:],
                            xt_blk[c][:, s0 + half * 512 : s0 + (half + 1) * 512],
                            start=(c == 0),
                            stop=(c == EC - 1),
                        )
                nc.vector.tensor_copy(dst[:], pj[:])
            # V natural tiles from V^T rows of the QV tile
            for u in range(SB // 128):
                j = sb * (SB // 128) + u
                pv = ps.tile([128, 64], bf16, tag="a")
                nc.tensor.transpose(
                    pv[:],
                    qv_tiles[sb][64:128, u * 128 : (u + 1) * 128],
                    eye[64:128, 64:128],
                )
                nc.vector.tensor_copy(v_tiles[j][:, 0:64], pv[:])

    # ---------------- attention ----------------
    with (
        tc.tile_pool(name="pt", bufs=NCH) as ptp,
        tc.tile_pool(name="eo", bufs=2) as eop,
    ):
        for m in range(NQB):
            qt = qv_tiles[m]  # Q^T for this sq block lives in rows 0:64
            ot = ps.tile([128, QB], f32, tag="b")  # O^T accumulator [65, QB]
            for j in range(NCH):
                ksb, ku = j // (SB // 128), j % (SB // 128)
                kslice = kt_tiles[ksb][0:64, ku * 128 : (ku + 1) * 128]
                st = ps.tile([128, QB], f32, tag="a")
                for half in range(QB // 512):
                    fsl = slice(half * 512, (half + 1) * 512)
                    nc.tensor.matmul(
                        st[:, fsl],
                        kslice,
                        qt[0:64, fsl],
                        start=True,
                        stop=True,
                    )
                pt = ptp.tile([128, QB], bf16, tag="pt")
                nc.scalar.activation(pt[:], st[:], EXP, scale=0.125)
                for half in range(QB // 512):
                    fsl = slice(half * 512, (half + 1) * 512)
                    nc.tensor.matmul(
                        ot[0:65, fsl],
                        v_tiles[j][:],
                        pt[:, fsl],
                        start=(j == 0),
                        stop=(j == NCH - 1),
                    )
            # epilogue: normalize in O^T layout, transpose, store
            rrow = eop.tile([1, QB], f32, tag="rrow")
            nc.vector.reciprocal(rrow[:], ot[64:65, :])
            rrep = eop.tile([64, QB], f32, tag="rrep")
            nc.gpsimd.partition_broadcast(rrep[:], rrow[:])
            onorm = eop.tile([64, QB], bf16, tag="onorm")
            nc.vector.tensor_mul(onorm[:], ot[0:64, :], rrep[:])
            tp = ps.tile([128, (QB // 128) * H], bf16, tag="a")
            for t in range(QB // 128):
                nc.tensor.transpose(
                    tp[:, t * H : (t + 1) * H],
                    onorm[:, t * 128 : (t + 1) * 128],
                    eye[0:64, 0:64],
                )
            ob = eop.tile([128, (QB // 128) * H], f32, tag="ob")
            nc.vector.tensor_copy(ob[:], tp[:])
            nc.sync.dma_start(
                out=out_d[m * QB : (m + 1) * QB, :].rearrange(
                    "(t p) h -> p t h", p=128
                ),
                in_=ob[:].rearrange("p (t h) -> p t h", h=H),
            )


def build_nc(iters=1):
    key = ("nc", iters)
    if key in _cache:
        return _cache[key]

    nc = bacc.Bacc("TRN2", target_bir_lowering=False, debug=False, num_devices=B)

    x_d = nc.dram_tensor("x", [S, E], bf16, kind="ExternalInput")
    wqv_d = nc.dram_tensor("wqv", [E, 128], bf16, kind="ExternalInput")
    wkk_d = nc.dram_tensor("wkk", [E, 128], bf16, kind="ExternalInput")
    eye_d = nc.dram_tensor("eye", [128, 128], bf16, kind="ExternalInput")
    ones_d = nc.dram_tensor("ones", [128, 1], bf16, kind="ExternalInput")
    out_d = nc.dram_tensor("out", [S, H], f32, kind="ExternalOutput")

    EC = E // 128

    with tile.TileContext(nc) as tc:
        with (
            tc.tile_pool(name="const", bufs=1) as cp,
            tc.tile_pool(name="persist", bufs=1) as pp,
            tc.tile_pool(name="ps", bufs=2, space="PSUM") as ps,
        ):
            eye = cp.tile([128, 128], bf16, tag="eye")
            nc.sync.dma_start(out=eye[:], in_=eye_d[:])
            wqv = []
            wkk = []
            for c in range(EC):
                wq_t = cp.tile([128, 128], bf16, tag=f"wqv{c}")
                wk_t = cp.tile([128, 128], bf16, tag=f"wkk{c}")
                nc.sync.dma_start(out=wq_t[:], in_=wqv_d[c * 128 : (c + 1) * 128, :])
                nc.sync.dma_start(out=wk_t[:], in_=wkk_d[c * 128 : (c + 1) * 128, :])
                wqv.append(wq_t)
                wkk.append(wk_t)

            consts = (eye, wqv, wkk, x_d, ones_d, out_d)
            for _ in range(iters):
                _emit_iteration(nc, tc, ps, pp, consts)

    nc.compile()
    _cache[key] = nc
    return nc


def make_in_maps(x, Wk, Wq, Wv):
    import ml_dtypes

    bf = ml_dtypes.bfloat16
    wqv = np.concatenate([Wq, Wv], axis=1).astype(bf)
    wkk = np.concatenate([Wk, Wk], axis=1).astype(bf)
    eye = np.eye(128, dtype=bf)
    x = np.asarray(x, np.float32).astype(bf)
    return [
        {
            "x": np.ascontiguousarray(x[i]),
            "wqv": wqv,
            "wkk": wkk,
            "eye": eye,
            "ones": np.ones((128, 1), dtype=bf),
        }
        for i in range(B)
    ]


def kernel(x, Wk, Wq, Wv):
    nc = build_nc()
    in_maps = make_in_maps(np.asarray(x), np.asarray(Wk), np.asarray(Wq), np.asarray(Wv))
    res = run_bass_kernel_spmd(nc, in_maps, core_ids=list(range(B)))
    return np.stack([res.results[i]["out"] for i in range(B)], axis=0)


# revision 12
# speedup vs baseline: 2.8496x; 2.8496x over previous
"""Single-head attention (batch 8, seq 4096, embed 1024, head 64) on 8 TRN2
NeuronCores, data-parallel over batch (one batch element per core).

Per-core pipeline (bf16 matmul operands, fp32 PSUM accumulation everywhere):
  1. Load x [4096, 1024] (pre-cast to bf16 on host), PE-transpose to
     x^T [1024, 4096] in s-blocks.
  2. Projections vs x^T (contraction on partitions):
       [Wq|Wv] pass -> QV tile [128, s]: rows 0:64 = Q^T, rows 64:128 = V^T
       [Wk|Wk] pass -> Kt tile [128, s]: rows 0:64 = K^T
     V^T slices are PE-transposed back to V-natural [128, 65] tiles with a
     ones column appended (softmax denominator trick).
  3. Attention in S^T orientation (scores transposed: [sk, sq]), so the
     P @ V contraction needs no transposes of the [4096, 4096] matrix:
       S^T chunk [128 sk, 1024 sq] = (K^T slice).T @ Q^T   (K=64 contraction)
       P^T = exp(S^T / 8) on ScalarE (fp32 PSUM -> bf16 SBUF); no max
       subtraction needed: scores ~N(0, 0.33^2), exp is safe in fp32.
       O^T [65, sq] += [V_j | 1].T @ P^T_j over all 32 sk chunks; row 64
       accumulates the softmax denominator (fp32 PSUM).
  4. Epilogue per sq block: reciprocal of the denominator row, gpsimd
     partition-broadcast, normalize in O^T layout, PE-transpose to
     O-natural, DMA out (fp32).
"""

import numpy as np

import concourse.bass as bass
import concourse.mybir as mybir
import concourse.tile as tile
from concourse import bacc
from concourse.bass_utils import run_bass_kernel_spmd

S = 4096  # sequence length (per core)
E = 1024  # embed dim
H = 64  # head size
B = 8  # batch == number of cores

SB = 1024  # prologue s-block
NSB = S // SB
QB = 1024  # attention sq-block
NQB = S // QB
CH = 128  # sk chunk
NCH = S // CH

f32 = mybir.dt.float32
bf16 = mybir.dt.bfloat16
EXP = mybir.ActivationFunctionType.Exp

_cache = {}


def _emit_iteration(nc, tc, ps, pp, consts):
    eye, wqv, wkk, x_d, ones_d, out_d = consts
    EC = E // 128

    qv_tiles = []  # [128, SB] per s-block: rows 0:64 Q^T, 64:128 V^T
    kt_tiles = []  # [128, SB] per s-block: rows 0:64 K^T
    for sb in range(NSB):
        qv_tiles.append(pp.tile([128, SB], bf16, tag=f"qv{sb}", name=f"qv{sb}"))
        kt_tiles.append(pp.tile([128, SB], bf16, tag=f"kt{sb}", name=f"kt{sb}"))
    v_tiles = []  # [128, 65] V natural + ones column, per sk chunk
    for j in range(NCH):
        v_tiles.append(pp.tile([128, 65], bf16, tag=f"v{j}", name=f"v{j}"))

    # ---------------- prologue: x^T (DMA transpose) + projections ----------------
    for j in range(NCH):
        nc.sync.dma_start(out=v_tiles[j][:, 64:65], in_=ones_d[:])
    with tc.tile_pool(name="xt", bufs=1) as xtp:
        xt_blk = []
        for c in range(EC):
            xt_c = xtp.tile([128, S], bf16, tag=f"xt{c}", name=f"xt{c}")
            nc.sync.dma_start_transpose(xt_c[:], x_d[:, c * 128 : (c + 1) * 128])
            xt_blk.append(xt_c)
        # K projections first across all s-blocks so attention unblocks early
        passes = [("kk", sb) for sb in range(NSB)] + [("qv", sb) for sb in range(NSB)]
        for kind, sb in passes:
            s0 = sb * SB
            w_tiles, dst = (
                (wkk, kt_tiles[sb]) if kind == "kk" else (wqv, qv_tiles[sb])
            )
            pj = ps.tile([128, SB], f32, tag="b")
            for half in range(SB // 512):
                for c in range(EC):
                    nc.tensor.matmul(
                        pj[:, half * 512 : (half + 1) * 512],
                        w_tiles[c][:],
                        xt_blk[c][:, s0 + half * 512 : s0 + (half + 1) * 512],
                        start=(c == 0),
                        stop=(c == EC - 1),
                    )
            nc.vector.tensor_copy(dst[:], pj[:])
            if kind == "qv":
                # V natural tiles from V^T rows of the QV tile
                for u in range(SB // 128):
                    j = sb * (SB // 128) + u
                    pv = ps.tile([128, 64], bf16, tag="a")
                    nc.tensor.transpose(
                        pv[:],
                        qv_tiles[sb][64:128, u * 128 : (u + 1) * 128],
                        eye[64:128, 64:128],
                    )
                    nc.vector.tensor_copy(v_tiles[j][:, 0:64], pv[:])

    # ---------------- attention ----------------
    with (
        tc.tile_pool(name="pt", bufs=8) as ptp,
        tc.tile_pool(name="eo", bufs=2) as eop,
    ):
        for m in range(NQB):
            qt = qv_tiles[m]  # Q^T for this sq block lives in rows 0:64
            ot = ps.tile([128, QB], f32, tag="b")  # O^T accumulator [65, QB]
            for j in range(NCH):
                ksb, ku = j // (SB // 128), j % (SB // 128)
                kslice = kt_tiles[ksb][0:64, ku * 128 : (ku + 1) * 128]
                st = ps.tile([128, QB], f32, tag="a")
                for half in range(QB // 512):
                    fsl = slice(half * 512, (half + 1) * 512)
                    nc.tensor.matmul(
                        st[:, fsl],
                        kslice,
                        qt[0:64, fsl],
                        start=True,
                        stop=True,
                    )
                pt = ptp.tile([128, QB], bf16, tag="pt")
                nc.scalar.activation(pt[:], st[:], EXP, scale=0.125)
                for half in range(QB // 512):
                    fsl = slice(half * 512, (half + 1) * 512)
                    nc.tensor.matmul(
                        ot[0:65, fsl],
                        v_tiles[j][:],
                        pt[:, fsl],
                        start=(j == 0),
                        stop=(j == NCH - 1),
                    )
            # epilogue: normalize in O^T layout, transpose, store
            rrow = eop.tile([1, QB], f32, tag="rrow")
            nc.vector.reciprocal(rrow[:], ot[64:65, :])
            rrep = eop.tile([64, QB], f32, tag="rrep")
            nc.gpsimd.partition_broadcast(rrep[:], rrow[:])
            onorm = eop.tile([64, QB], bf16, tag="onorm")
            nc.vector.tensor_mul(onorm[:], ot[0:64, :], rrep[:])
            tp = ps.tile([128, (QB // 128) * H], bf16, tag="a")
            for t in range(QB // 128):
                nc.tensor.transpose(
                    tp[:, t * H : (t + 1) * H],
                    onorm[:, t * 128 : (t + 1) * 128],
                    eye[0:64, 0:64],
                )
            ob = eop.tile([128, (QB // 128) * H], f32, tag="ob")
            nc.vector.tensor_copy(ob[:], tp[:])
            nc.sync.dma_start(
                out=out_d[m * QB : (m + 1) * QB, :].rearrange(
                    "(t p) h -> p t h", p=128
                ),
                in_=ob[:].rearrange("p (t h) -> p t h", h=H),
            )


def build_nc(iters=1):
    key = ("nc", iters)
    if key in _cache:
        return _cache[key]

    nc = bacc.Bacc("TRN2", target_bir_lowering=False, debug=False, num_devices=B)

    x_d = nc.dram_tensor("x", [S, E], bf16, kind="ExternalInput")
    wqv_d = nc.dram_tensor("wqv", [E, 128], bf16, kind="ExternalInput")
    wkk_d = nc.dram_tensor("wkk", [E, 128], bf16, kind="ExternalInput")
    eye_d = nc.dram_tensor("eye", [128, 128], bf16, kind="ExternalInput")
    ones_d = nc.dram_tensor("ones", [128, 1], bf16, kind="ExternalInput")
    out_d = nc.dram_tensor("out", [S, H], f32, kind="ExternalOutput")

    EC = E // 128

    with tile.TileContext(nc) as tc:
        with (
            tc.tile_pool(name="const", bufs=1) as cp,
            tc.tile_pool(name="persist", bufs=1) as pp,
            tc.tile_pool(name="ps", bufs=2, space="PSUM") as ps,
        ):
            eye = cp.tile([128, 128], bf16, tag="eye")
            nc.sync.dma_start(out=eye[:], in_=eye_d[:])
            wqv = []
            wkk = []
            for c in range(EC):
                wq_t = cp.tile([128, 128], bf16, tag=f"wqv{c}")
                wk_t = cp.tile([128, 128], bf16, tag=f"wkk{c}")
                nc.sync.dma_start(out=wq_t[:], in_=wqv_d[c * 128 : (c + 1) * 128, :])
                nc.sync.dma_start(out=wk_t[:], in_=wkk_d[c * 128 : (c + 1) * 128, :])
                wqv.append(wq_t)
                wkk.append(wk_t)

            consts = (eye, wqv, wkk, x_d, ones_d, out_d)
            for _ in range(iters):
                _emit_iteration(nc, tc, ps, pp, consts)

    nc.compile()
    _cache[key] = nc
    return nc


def make_in_maps(x, Wk, Wq, Wv):
    import ml_dtypes

    bf = ml_dtypes.bfloat16
    wqv = np.concatenate([Wq, Wv], axis=1).astype(bf)
    wkk = np.concatenate([Wk, Wk], axis=1).astype(bf)
    eye = np.eye(128, dtype=bf)
    x = np.asarray(x, np.float32).astype(bf)
    return [
        {
            "x": np.ascontiguousarray(x[i]),
            "wqv": wqv,
            "wkk": wkk,
            "eye": eye,
            "ones": np.ones((128, 1), dtype=bf),
        }
        for i in range(B)
    ]


def kernel(x, Wk, Wq, Wv):
    nc = build_nc()
    in_maps = make_in_maps(np.asarray(x), np.asarray(Wk), np.asarray(Wq), np.asarray(Wv))
    res = run_bass_kernel_spmd(nc, in_maps, core_ids=list(range(B)))
    return np.stack([res.results[i]["out"] for i in range(B)], axis=0)
